# revision 70
# baseline (speedup 1.0000x reference)
"""AWQ 4-bit quantized linear layer for Trainium2, tensor-parallel over 8 NeuronCores.

Computes y = x @ dequant(qweight, scales).T + bias where
  x: (4096, 4096) f32, qweight: (12288, 512) int32 (8 x 4-bit nibbles per word,
  high nibble first), scales: (12288, 32) f32 (group size 128), bias: (12288,) f32.

Sharding: column-parallel -- qweight/scales/bias sharded along out_features across
8 cores, x replicated. Each core computes y[:, shard] = [4096, 1536]; host concat.

Per-core device kernel (centered-fp8 DoubleRow + rank-32 mean term):
  - Weights are CENTERED: nib = 7.5 + c with c in [-7.5, 7.5]. Weights stay
    packed on the wire (4 bits/weight); the vector engine extracts nibbles
    (int16 shift/and, 2x rate) and dequantizes fp8 planes via
    (nib - 7.5) * bf16(s*256) -> fp8e4. Centering shrinks E[w^2] by 3.65x so
    fp8 rounding noise fits 28 of 32 K-blocks in fp8 DoubleRow (14 matmuls,
    K=256 each, 2x MAC rate). [Measured dead ends: int8/int16 de-interleaved
    host planes triple the weight DMA and starve the PE at startup; gpsimd
    ALU is ~2x slower than vector and rejects strided operands and
    shift/and tensor_scalar, so dequant stays on the vector engine.]
  - The 7.5-mean part is EXACT: a rank-32 matmul per out-tile adds
    xsum[t,g] * (7.5 * s[g,o]) where xsum = per-scale-group sums of exact f32 x
    over the fp8 planes (computed on host, bf16). Replaces the old rank-16 dx
    correction at the same cost (one K<=128 matmul per PSUM group).
  - 4 K-blocks (kb0 planes j0..3) stay bf16 (uncentered, exact path) to hold
    rel-err ~1.94e-2 < 2e-2 on the seeded inputs.
  - All scales pre-multiplied by 256 on the host, stored bf16 so every dequant
    op is all-16-bit (2x DVE rate); PSUM holds 256*y. bias*256 rides K-row 32
    of the mean-term matmul (against an all-ones xg row), so eviction is a pure
    x(1/256) scale running on the otherwise-idle scalar engine.
  - Matmul loops o-slice-outer / t-tile-inner; slice-0 startup runs R0 t-tile
    groups pair-major so the PE starts as soon as the first weight pair is
    dequantized. Later slices dequantize on the DVE interleaved with PSUM
    evictions while the PE streams.
  - A burst of full-width dummy matmuls at kernel start warms the PE clock
    while the first weight DMAs spin up.

K-index permutation: K-block kk=(kb,j) holds i = 8*(128*kb+p)+j for p=0..127;
the x operand is pre-permuted on the host to match, so the contraction is
consistent (order within K is irrelevant to the dot product). Scale group of
element (kb, p, j) is g = 8*kb + p//16, independent of j.
"""

from contextlib import ExitStack

import numpy as np
import ml_dtypes

O, I, T = 12288, 4096, 4096
NCORES = 8
OS = O // NCORES          # 1536 out features per core
KB = 4                    # packed-word partition blocks (512 / 128)
NKB_BF = 4                # bf16 K-blocks: kb0 planes j0..3 (kk 0..3)
NKB = 32                  # K-blocks of 128 (4096 / 128)
NPAIR = 14                # fp8 DoubleRow pairs (28 K-blocks)
# pair q -> (kkA, kkB). kb0: (4,5),(6,7); kb1..3: (8kb+jj, 8kb+jj+4).
PAIRS = [(4, 5), (6, 7)] + [(8 * kb + jj, 8 * kb + jj + 4)
                            for kb in range(1, 4) for jj in range(4)]
NCH = 32                  # mean-term channels: all 32 scale groups
TT = T // 128             # 32 t-tiles
OT = OS // 512            # 3 o-slices per core
N_WARM = 122              # dummy matmuls to warm the PE clock
R0 = 6                    # t-tile groups interleaved pair-major during slice-0
                          # (R0=7 measured 513us: 7 ps banks + warmup bank
                          # leaves no spare PSUM bank and serializes groups)
WSCALE = 256.0            # scale fold so fp8 weights stay in e4m3 normal range
CENTER = 7.5              # nibble centering offset

_nc_cache = None
LAST_RESULTS = None

# (kb, j) -> ("bf", kk) or ("f8", pair, half)
_DEQ_DEST = {}
for _j in range(NKB_BF):
    _DEQ_DEST[(0, _j)] = ("bf", _j)
for _q, (_a, _b) in enumerate(PAIRS):
    _DEQ_DEST[(_a // 8, _a % 8)] = ("f8", _q, 0)
    _DEQ_DEST[(_b // 8, _b % 8)] = ("f8", _q, 1)


def _build_module():
    import concourse.tile as tile
    from concourse import bacc, mybir

    nc = bacc.Bacc("TRN2", target_bir_lowering=False, debug=False,
                   num_devices=NCORES)

    xp = nc.dram_tensor("xp", [TT, 128, NKB_BF * 128], mybir.dt.bfloat16,
                        kind="ExternalInput").ap()
    xp8 = nc.dram_tensor("xp8", [TT, 128, NPAIR * 256], mybir.dt.float8e4,
                         kind="ExternalInput").ap()
    # per (ot, kb): 512 packed int32 nibble words then 512 bf16 scales
    qws = nc.dram_tensor("qws", [OT, KB, 128, 768], mybir.dt.int32,
                         kind="ExternalInput").ap()
    # mean-term operands K-padded to 64 rows (33 live: 32 group channels +
    # the bias/ones row) -- half the DMA bytes of 128-row padding, so they
    # land on the scalar queue right as the PE warmup ends
    xg = nc.dram_tensor("xg", [64, T], mybir.dt.bfloat16,
                        kind="ExternalInput").ap()
    s75 = nc.dram_tensor("s75", [64, OS], mybir.dt.bfloat16,
                         kind="ExternalInput").ap()
    y = nc.dram_tensor("y", [TT, 128, OS], mybir.dt.float32,
                       kind="ExternalOutput").ap()

    ts = lambda i, s: slice(i * s, (i + 1) * s)

    with tile.TileContext(nc) as tc:
        with ExitStack() as ctx:
            qs_pool = ctx.enter_context(tc.tile_pool(name="qs", bufs=6))
            nib_pool = ctx.enter_context(tc.tile_pool(name="nib", bufs=8))
            w_pool = ctx.enter_context(tc.tile_pool(name="wd", bufs=OT * NKB_BF))
            w8_pool = ctx.enter_context(tc.tile_pool(name="w8", bufs=OT * NPAIR))
            x_pool = ctx.enter_context(tc.tile_pool(name="xt", bufs=3))
            x8_pool = ctx.enter_context(tc.tile_pool(name="x8", bufs=4))
            xc_pool = ctx.enter_context(tc.tile_pool(name="xc", bufs=R0))
            xc8_pool = ctx.enter_context(tc.tile_pool(name="xc8", bufs=R0))
            c_pool = ctx.enter_context(tc.tile_pool(name="cst", bufs=1))
            o_pool = ctx.enter_context(tc.tile_pool(name="out", bufs=3))
            ps_pool = ctx.enter_context(
                tc.tile_pool(name="ps", bufs=R0, space="PSUM"))
            psw_pool = ctx.enter_context(
                tc.tile_pool(name="psw", bufs=1, space="PSUM"))

            # --- PE warm-up: dummy matmuls on a zeroed scratch tile, spanning
            # the DMA-queue spin-up (~12us) at low energy ---
            scratch = c_pool.tile([128, 128], mybir.dt.bfloat16)
            nc.gpsimd.memset(scratch[:], 0.0)
            psw = psw_pool.tile([128, 64], mybir.dt.float32)
            for _ in range(N_WARM):
                nc.tensor.matmul(psw[:], scratch[:], scratch[:, 0:64],
                                 start=True, stop=True)

            # --- slice-0 packed weights + scales first on the sync queue ---
            qw16 = {}   # (ot, kb) -> int16 view [128, 1024]
            sc_t = {}   # (ot, kb) -> bf16 view [128, 512]

            def emit_wdma(ot, kb_order=range(KB), split_first=False):
                # one fused DMA per (ot, kb): packed words (cols 0-511) and
                # bf16 scale bits (cols 512-767) land together. split_first
                # lands the first kb's packed words ahead of its scales so
                # the first TENSOR_SCALAR can start sooner.
                for idx, kb in enumerate(kb_order):
                    q = qs_pool.tile([128, 768], mybir.dt.int32, tag="qs")
                    if split_first and idx == 0:
                        nc.sync.dma_start(q[:, 0:512], qws[ot, kb, :, 0:512])
                        # the first kb's scales ride the scalar queue (131KB,
                        # behind only xg/s75) and land ~4us before the sync
                        # queue would deliver them
                        nc.scalar.dma_start(q[:, 512:768],
                                            qws[ot, kb, :, 512:768])
                    else:
                        nc.sync.dma_start(q[:], qws[ot, kb])
                    qw16[(ot, kb)] = q[:, 0:512].bitcast(mybir.dt.int16)
                    sc_t[(ot, kb)] = q[:, 512:768].bitcast(mybir.dt.bfloat16)

            # mean-term operands FIRST on the scalar queue (0.6MB total:
            # lands right as warmup ends, so the corr-first matmuls bridge
            # the gap until the first dequantized weight pair)
            xg_t = c_pool.tile([64, T], mybir.dt.bfloat16)
            s75_t = c_pool.tile([64, OS], mybir.dt.bfloat16)
            nc.scalar.dma_start(xg_t[:], xg)
            nc.scalar.dma_start(s75_t[:], s75)
            # slice-0 startup consumes pairs q0.. in order; q0/q1 are kb0.
            emit_wdma(0, kb_order=(0, 1, 2, 3), split_first=True)

            # --- o-sliced dequantization op streams (vector engine) ---
            # K-block kk = 8*kb + j; plane j lives in the int16 nibble-pair
            # extracted with shift s = 12 - 4*(j%4) at column parity
            # e = 1 (odd, j<4) or 0 (even, j>=4).
            wd = [[None] * NKB_BF for _ in range(OT)]
            w8 = [[None] * NPAIR for _ in range(OT)]

            DEFAULT_SCHED = tuple((kb, tuple(range(8))) for kb in range(KB))
            # slice 0: kb0 fp8 planes first (pairs q0,q1 feed the PE right
            # after warm-up), bf16 planes after; kb1..3 in (j, j+4) order so
            # each pair completes right after its nibble tile is extracted.
            SLICE0_SCHED = (
                (0, (4, 5, 6, 7, 0, 1, 2, 3)),
                (1, (0, 4, 1, 5, 2, 6, 3, 7)),
                (2, (0, 4, 1, 5, 2, 6, 3, 7)),
                (3, (0, 4, 1, 5, 2, 6, 3, 7)),
            )

            def slice_deq_ops(ot, sched=DEFAULT_SCHED):
                """Yield thunks emitting slice `ot`'s dequant (4xTS + 8 ops per kb)."""
                nibs_by_kb = {}

                def emit_ts(kb, nibs, shifts):
                    def go():
                        for s in shifts:
                            nib = nib_pool.tile([128, 1024], mybir.dt.int16,
                                                tag="nib", name=f"nib{s}")
                            nc.vector.tensor_scalar(
                                nib[:], qw16[(ot, kb)], s, 15,
                                op0=mybir.AluOpType.logical_shift_right,
                                op1=mybir.AluOpType.bitwise_and,
                            )
                            nibs[s] = nib
                    return go

                def emit_tt(j, kb, nibs):
                    def go():
                        s_ = 12 - 4 * (j % 4)
                        e = 1 if j < 4 else 0
                        pair = nibs[s_][:].rearrange("p (o e) -> p o e", e=2)
                        dest = _DEQ_DEST[(kb, j)]
                        if dest[0] == "bf":
                            w = w_pool.tile([128, 512], mybir.dt.bfloat16,
                                            tag="w")
                            nc.vector.tensor_tensor(
                                w[:], pair[:, :, e], sc_t[(ot, kb)],
                                op=mybir.AluOpType.mult)
                            wd[ot][dest[1]] = w
                        else:
                            _, q_, half = dest
                            if w8[ot][q_] is None:
                                w8[ot][q_] = w8_pool.tile(
                                    [128, 1024], mybir.dt.float8e4,
                                    tag="w8", name=f"w8_{ot}_{q_}")
                            # centered dequant: (nib - 7.5) * bf16(s*256)
                            nc.vector.scalar_tensor_tensor(
                                w8[ot][q_][:, ts(half, 512)],
                                pair[:, :, e], -CENTER, sc_t[(ot, kb)],
                                op0=mybir.AluOpType.add,
                                op1=mybir.AluOpType.mult)
                    return go

                for kb, js in sched:
                    if kb not in nibs_by_kb:
                        nibs_by_kb[kb] = {}
                        yield emit_ts(kb, nibs_by_kb[kb], (12, 8, 4, 0))
                    for j in js:
                        yield emit_tt(j, kb, nibs_by_kb[kb])

            def evict(ot, tt, ps):
                # bias already accumulated in PSUM via the mean-term matmul's
                # ones row; eviction is a pure scale on the idle scalar engine.
                # Output DMAs alternate queues so back-to-back evictions (and
                # the final pair at the kernel tail) don't serialize.
                ob = o_pool.tile([128, 512], mybir.dt.float32, tag="ob")
                nc.scalar.mul(ob[:], ps[:], 1.0 / WSCALE)
                nc.sync.dma_start(y[tt, :, ts(ot, 512)], ob[:])

            def corr_mm(ps, ot, tt, stop=False):
                nc.tensor.matmul(ps[:], xg_t[:, ts(tt, 128)],
                                 s75_t[:, ts(ot, 512)],
                                 start=False, stop=stop)

            def dr_mm1(ps, xt8_ap, ot, q_, start=False, stop=False):
                from concourse import mybir as mb
                lhs3 = xt8_ap[:, ts(q_, 256)].rearrange(
                    "p (two m) -> p two m", two=2)
                rhs3 = w8[ot][q_][:].rearrange(
                    "p (two o) -> p two o", two=2)
                nc.tensor.matmul(
                    ps[:], lhs3, rhs3, start=start, stop=stop,
                    perf_mode=mb.MatmulPerfMode.DoubleRow,
                )

            def dr_mms(ps, xt8_ap, ot, start=False):
                for q_ in range(NPAIR):
                    dr_mm1(ps, xt8_ap, ot, q_, start=(start and q_ == 0))

            # --- matmul: o-slice outer, t-tile inner ---
            # The first R0 t-tile groups of slice 0 run pair-major across R0
            # PSUM banks: each weight pair feeds R0 matmuls the moment the
            # dequant engines produce it, keeping the in-order PE busy during
            # slice-0 dequantization.
            xcs = [None] * R0
            xc8s = [None] * R0
            pss = []
            # gpsimd DMA issues FIRST in its program order (it also runs
            # dequant ops now); fp8 x first (DR phase starts the instant
            # warmup ends), then the small bf16 chunks.
            for g in range(R0):
                xc8 = xc8_pool.tile([128, NPAIR * 256], mybir.dt.float8e4,
                                    tag="xc8", name=f"xc8_{g}")
                nc.gpsimd.dma_start(xc8[:], xp8[g])
                xc8s[g] = xc8
            for g in range(R0):
                xc = xc_pool.tile([128, NKB_BF * 128], mybir.dt.bfloat16,
                                  tag="xc", name=f"xc{g}")
                nc.gpsimd.dma_start(xc[:], xp[g])
                xcs[g] = xc
            # slice-0 dequant emitted after the DMA issues above
            for op in slice_deq_ops(0, SLICE0_SCHED):
                op()
            for g in range(R0):
                pss.append(ps_pool.tile([128, 512], mybir.dt.float32,
                                        tag="ps", name=f"ps0_{g}"))
            # the mean-term operands land early on the scalar queue: the
            # rank-33 matmuls give the PE real work while the sync queue is
            # still delivering the first weight tiles
            for g in range(R0):
                nc.tensor.matmul(pss[g][:], xg_t[:, ts(g, 128)],
                                 s75_t[:, 0:512], start=True, stop=False)
            for q_ in range(NPAIR):
                for g in range(R0):
                    dr_mm1(pss[g], xc8s[g][:], 0, q_)
            for kk in range(NKB_BF):
                for g in range(R0):
                    nc.tensor.matmul(
                        pss[g][:], xcs[g][:, ts(kk, 128)], wd[0][kk][:],
                        start=False, stop=(kk == NKB_BF - 1),
                    )
            for g in range(R0):
                evict(0, g, pss[g])

            def bf_mms(ps, xt_bf, ot):
                # groups are started by the DR chain; never start here
                for kk in range(NKB_BF):
                    nc.tensor.matmul(
                        ps[:], xt_bf[:, ts(kk, 128)], wd[ot][kk][:],
                        start=False, stop=False,
                    )

            def load_x(tt, eng=None):
                # balance x traffic across both DMA queues: gpsimd saturates
                # during the slice-0 pairs loop if it carries every tile
                eng = eng or nc.gpsimd
                xt_bf = x_pool.tile([128, NKB_BF * 128], mybir.dt.bfloat16,
                                    tag="x", name=f"xt{tt}")
                eng.dma_start(xt_bf[:], xp[tt])
                xt8 = x8_pool.tile([128, NPAIR * 256], mybir.dt.float8e4,
                                   tag="x8t", name=f"xt8{tt}")
                nc.gpsimd.dma_start(xt8[:], xp8[tt])
                return xt_bf, xt8

            # --- slice 0, tiles R0..TT-1 in pairs (one DR bubble per pair),
            # interleaving slice 1+2 dequant between evictions ---
            emit_wdma(1)
            pending = list(slice_deq_ops(1)) + list(slice_deq_ops(2))
            n_single = (TT - R0) % 2
            n_pairs = (TT - R0) // 2
            per_pair = (len(pending) + n_pairs - 1) // n_pairs
            for pi in range(n_pairs):
                if pi == 3:
                    # slice-2 packed weights aren't consumed until the
                    # second half of the pairs loop; keep sync free early
                    emit_wdma(2)
                ta, tb = R0 + 2 * pi, R0 + 2 * pi + 1
                xa, xa8 = load_x(ta)
                xb, xb8 = load_x(tb, eng=nc.sync)
                psa = ps_pool.tile([128, 512], mybir.dt.float32, tag="ps",
                                   name=f"psa{pi}")
                psb = ps_pool.tile([128, 512], mybir.dt.float32, tag="ps",
                                   name=f"psb{pi}")
                # every PSUM group starts with its DR chain and every
                # normal-section entry follows a DR MM: all group starts and
                # mode transitions are the measured-free kind
                dr_mms(psa, xa8[:], 0, start=True)
                bf_mms(psa, xa, 0)
                corr_mm(psa, 0, ta, stop=True)
                dr_mms(psb, xb8[:], 0, start=True)
                bf_mms(psb, xb, 0)
                corr_mm(psb, 0, tb, stop=True)
                evict(0, ta, psa)
                evict(0, tb, psb)
                for _ in range(per_pair):
                    if pending:
                        pending.pop(0)()
            assert not pending
            for tt in range(TT - n_single, TT):
                # odd leftover slice-0 tile when (TT - R0) is odd
                xt_bf, xt8 = load_x(tt)
                pst = ps_pool.tile([128, 512], mybir.dt.float32, tag="ps",
                                   name=f"pst{tt}")
                dr_mms(pst, xt8[:], 0, start=True)
                bf_mms(pst, xt_bf, 0)
                corr_mm(pst, 0, tt, stop=True)
                evict(0, tt, pst)

            # --- slices 1+2 fused per t-tile: x loaded once, DR batched ---
            for tt in range(TT):
                xt_bf, xt8 = load_x(tt, eng=(nc.sync if tt % 2 else None))
                ps1 = ps_pool.tile([128, 512], mybir.dt.float32, tag="ps",
                                   name=f"ps1_{tt}")
                ps2 = ps_pool.tile([128, 512], mybir.dt.float32, tag="ps",
                                   name=f"ps2_{tt}")
                dr_mms(ps1, xt8[:], 1, start=True)
                bf_mms(ps1, xt_bf, 1)
                corr_mm(ps1, 1, tt, stop=True)
                dr_mms(ps2, xt8[:], 2, start=True)
                bf_mms(ps2, xt_bf, 2)
                corr_mm(ps2, 2, tt, stop=True)
                evict(1, tt, ps1)
                evict(2, tt, ps2)

    nc.compile()
    return nc


def _prep_inputs(x, qweight, scales, bias):
    bf16 = ml_dtypes.bfloat16
    fp8 = ml_dtypes.float8_e4m3
    # x -> K-permuted lhsT layout: XKK[tt, p, kk, m] = x[128*tt+m, i(kk, p)]
    # with i(kk=(kb,j), p) = 8*(128*kb + p) + j.
    xb = np.ascontiguousarray(x.T)                            # [I, T] f32
    xb = xb.reshape(KB, 128, 8, T).transpose(0, 2, 1, 3)      # [kb, j, p, t]
    xb = xb.reshape(NKB, 128, TT, 128).transpose(2, 1, 0, 3)  # [tt, p, kk, m]
    xkk = np.ascontiguousarray(xb)                            # f32
    # bf16 part: kk 0..NKB_BF-1
    xp = xkk[:, :, :NKB_BF, :].astype(bf16).reshape(TT, 128, NKB_BF * 128)
    # fp8 DoubleRow pairs
    x8l = [xkk[:, :, [a, b], :] for (a, b) in PAIRS]          # [tt,p,2,128] each
    x8 = np.stack(x8l, axis=2)                                # [tt,p,np,2,128]
    xp8 = np.ascontiguousarray(x8).astype(fp8).reshape(TT, 128, NPAIR * 256)

    # qweight -> [o-slice, kb, p, 512] per-core shards (packed words -- 4
    # bits/weight on the wire; nibble extraction on-device on the vector
    # engine where int16 shift/and runs at 2x rate)
    qwt = np.ascontiguousarray(qweight.T).reshape(KB, 128, O)

    # SC[kb, p, o] = bf16(scales[o, 8*kb + p//16] * WSCALE). The mean term
    # uses the SAME bf16 scale so the centering offset cancels exactly.
    st = np.ascontiguousarray(scales.T) * np.float32(WSCALE)  # [32, O]
    scp = np.repeat(st.reshape(KB, 8, O), 16, axis=1).astype(bf16)

    # --- rank-33 mean term (host side) ---
    # channels 0..31: exact x group sums over fp8 planes; channel 32: ones
    # row carrying bias*WSCALE. ch = 8*kb + G == scale group.
    xv = x.reshape(T, KB, 128, 8)                             # [t, kb, p, j]
    xg_host = np.zeros((64, T), dtype=bf16)                   # K-padded
    for kb in range(KB):
        js = [j for j in range(8) if _DEQ_DEST[(kb, j)][0] == "f8"]
        part = xv[:, kb, :, js].sum(axis=0)                   # [t, p]
        gsum = part.reshape(T, 8, 16).sum(axis=2)             # [t, G]
        xg_host[8 * kb:8 * kb + 8, :] = (
            np.ascontiguousarray(gsum.T).astype(bf16))
    xg_host[NCH, :] = bf16(1.0)
    # s75[ch, o] = 7.5 * bf16(scales * WSCALE); row NCH = bias * WSCALE
    scbf = np.ascontiguousarray(st.astype(bf16).astype(np.float32))
    s75_host = np.zeros((64, O), dtype=bf16)                  # K-padded
    s75_host[:NCH] = (np.float32(CENTER) * scbf).astype(bf16)
    s75_host[NCH] = (bias * np.float32(WSCALE)).astype(bf16)

    in_maps = []
    for c in range(NCORES):
        sl = slice(c * OS, (c + 1) * OS)
        qc = qwt[:, :, sl].reshape(KB, 128, OT, 512).transpose(2, 0, 1, 3)
        scc = scp[:, :, sl].reshape(KB, 128, OT, 512).transpose(2, 0, 1, 3)
        scc32 = np.ascontiguousarray(scc).view(np.uint16).reshape(
            OT, KB, 128, 512).view(np.uint32).view(np.int32)  # [.., 256]
        fused = np.concatenate([np.ascontiguousarray(qc), scc32], axis=3)
        in_maps.append({
            "xp": xp,
            "xp8": xp8,
            "qws": fused,
            "xg": xg_host,
            "s75": np.ascontiguousarray(s75_host[:, sl]),
        })
    return in_maps


def kernel(x, qweight, scales, bias):
    global _nc_cache, LAST_RESULTS
    from concourse.bass_utils import run_bass_kernel_spmd

    x = np.asarray(x, dtype=np.float32)
    qweight = np.asarray(qweight, dtype=np.int32)
    scales = np.asarray(scales, dtype=np.float32)
    bias = np.asarray(bias, dtype=np.float32)

    if _nc_cache is None:
        _nc_cache = _build_module()
    nc = _nc_cache

    in_maps = _prep_inputs(x, qweight, scales, bias)
    res = None
    for attempt in range(3):
        try:
            res = run_bass_kernel_spmd(nc, in_maps,
                                       core_ids=list(range(NCORES)))
            break
        except Exception:
            if attempt == 2:
                raise
    LAST_RESULTS = res
    return np.concatenate(
        [r["y"].reshape(T, OS) for r in res.results], axis=1)


# revision 74
# speedup vs baseline: 1.0553x; 1.0553x over previous
"""AWQ 4-bit quantized linear layer for Trainium2, tensor-parallel over 8 NeuronCores.

Computes y = x @ dequant(qweight, scales).T + bias where
  x: (4096, 4096) f32, qweight: (12288, 512) int32 (8 x 4-bit nibbles per word,
  high nibble first), scales: (12288, 32) f32 (group size 128), bias: (12288,) f32.

Sharding: column-parallel -- qweight/scales/bias sharded along out_features across
8 cores, x replicated. Each core computes y[:, shard] = [4096, 1536]; host concat.

Per-core device kernel (centered-fp8 DoubleRow + rank-32 mean term):
  - Weights are CENTERED: nib = 7.5 + c with c in [-7.5, 7.5]. Weights stay
    packed on the wire (4 bits/weight); the vector engine extracts nibbles
    (int16 shift/and, 2x rate) and dequantizes fp8 planes via
    (nib - 7.5) * bf16(s*256) -> fp8e4. Centering shrinks E[w^2] by 3.65x so
    fp8 rounding noise fits 28 of 32 K-blocks in fp8 DoubleRow (14 matmuls,
    K=256 each, 2x MAC rate). [Measured dead ends: int8/int16 de-interleaved
    host planes triple the weight DMA and starve the PE at startup; gpsimd
    ALU is ~2x slower than vector and rejects strided operands and
    shift/and tensor_scalar, so dequant stays on the vector engine.]
  - The 7.5-mean part is EXACT: a rank-32 matmul per out-tile adds
    xsum[t,g] * (7.5 * s[g,o]) where xsum = per-scale-group sums of exact f32 x
    over the fp8 planes (computed on host, bf16). Replaces the old rank-16 dx
    correction at the same cost (one K<=128 matmul per PSUM group).
  - 4 K-blocks (kb0 planes j0..3) stay bf16 (uncentered, exact path) to hold
    rel-err ~1.94e-2 < 2e-2 on the seeded inputs.
  - All scales pre-multiplied by 256 on the host, stored bf16 so every dequant
    op is all-16-bit (2x DVE rate); PSUM holds 256*y. bias*256 rides K-row 32
    of the mean-term matmul (against an all-ones xg row), so eviction is a pure
    x(1/256) scale running on the otherwise-idle scalar engine.
  - Matmul loops o-slice-outer / t-tile-inner; slice-0 startup runs R0 t-tile
    groups pair-major so the PE starts as soon as the first weight pair is
    dequantized. Later slices dequantize on the DVE interleaved with PSUM
    evictions while the PE streams.
  - A burst of full-width dummy matmuls at kernel start warms the PE clock
    while the first weight DMAs spin up.

K-index permutation: K-block kk=(kb,j) holds i = 8*(128*kb+p)+j for p=0..127;
the x operand is pre-permuted on the host to match, so the contraction is
consistent (order within K is irrelevant to the dot product). Scale group of
element (kb, p, j) is g = 8*kb + p//16, independent of j.
"""

from contextlib import ExitStack

import numpy as np
import ml_dtypes

O, I, T = 12288, 4096, 4096
NCORES = 8
OS = O // NCORES          # 1536 out features per core
KB = 4                    # packed-word partition blocks (512 / 128)
NKB_BF = 4                # bf16 K-blocks: kb0 planes j0..3 (kk 0..3)
NKB = 32                  # K-blocks of 128 (4096 / 128)
NPAIR = 14                # fp8 DoubleRow pairs (28 K-blocks)
# pair q -> (kkA, kkB). kb0: (4,5),(6,7); kb1..3: (8kb+jj, 8kb+jj+4).
PAIRS = [(4, 5), (6, 7)] + [(8 * kb + jj, 8 * kb + jj + 4)
                            for kb in range(1, 4) for jj in range(4)]
NCH = 32                  # mean-term channels: all 32 scale groups
TT = T // 128             # 32 t-tiles
OT = OS // 512            # 3 o-slices per core
N_WARM = 122              # dummy matmuls to warm the PE clock
R0 = 6                    # t-tile groups interleaved pair-major during slice-0
                          # (R0=7 measured 513us: 7 ps banks + warmup bank
                          # leaves no spare PSUM bank and serializes groups)
WSCALE = 256.0            # scale fold so fp8 weights stay in e4m3 normal range
CENTER = 7.5              # nibble centering offset

_nc_cache = None
LAST_RESULTS = None

# (kb, j) -> ("bf", kk) or ("f8", pair, half)
_DEQ_DEST = {}
for _j in range(NKB_BF):
    _DEQ_DEST[(0, _j)] = ("bf", _j)
for _q, (_a, _b) in enumerate(PAIRS):
    _DEQ_DEST[(_a // 8, _a % 8)] = ("f8", _q, 0)
    _DEQ_DEST[(_b // 8, _b % 8)] = ("f8", _q, 1)


def _build_module():
    import concourse.tile as tile
    from concourse import bacc, mybir

    nc = bacc.Bacc("TRN2", target_bir_lowering=False, debug=False,
                   num_devices=NCORES)

    xp = nc.dram_tensor("xp", [TT, 128, NKB_BF * 128], mybir.dt.bfloat16,
                        kind="ExternalInput").ap()
    xp8 = nc.dram_tensor("xp8", [TT, 128, NPAIR * 256], mybir.dt.float8e4,
                         kind="ExternalInput").ap()
    # per (ot, kb): 512 packed int32 nibble words then 512 bf16 scales
    qws = nc.dram_tensor("qws", [OT, KB, 128, 768], mybir.dt.int32,
                         kind="ExternalInput").ap()
    # mean-term operands ship as 64 rows (33 live) -- half the DMA bytes --
    # but land in 128-row SBUF tiles (upper half memset to 0) so the matmul
    # keeps K=128: a K=64 lhsT changes the PE tile_size per group and was
    # measured +20us across the 96 groups
    xg = nc.dram_tensor("xg", [64, T], mybir.dt.bfloat16,
                        kind="ExternalInput").ap()
    s75 = nc.dram_tensor("s75", [64, OS], mybir.dt.bfloat16,
                         kind="ExternalInput").ap()
    y = nc.dram_tensor("y", [TT, 128, OS], mybir.dt.float32,
                       kind="ExternalOutput").ap()

    ts = lambda i, s: slice(i * s, (i + 1) * s)

    with tile.TileContext(nc) as tc:
        with ExitStack() as ctx:
            qs_pool = ctx.enter_context(tc.tile_pool(name="qs", bufs=6))
            nib_pool = ctx.enter_context(tc.tile_pool(name="nib", bufs=8))
            w_pool = ctx.enter_context(tc.tile_pool(name="wd", bufs=OT * NKB_BF))
            w8_pool = ctx.enter_context(tc.tile_pool(name="w8", bufs=OT * NPAIR))
            x_pool = ctx.enter_context(tc.tile_pool(name="xt", bufs=3))
            x8_pool = ctx.enter_context(tc.tile_pool(name="x8", bufs=4))
            xc_pool = ctx.enter_context(tc.tile_pool(name="xc", bufs=R0))
            xc8_pool = ctx.enter_context(tc.tile_pool(name="xc8", bufs=R0))
            c_pool = ctx.enter_context(tc.tile_pool(name="cst", bufs=1))
            o_pool = ctx.enter_context(tc.tile_pool(name="out", bufs=3))
            ps_pool = ctx.enter_context(
                tc.tile_pool(name="ps", bufs=R0, space="PSUM"))
            psw_pool = ctx.enter_context(
                tc.tile_pool(name="psw", bufs=1, space="PSUM"))

            # --- PE warm-up: dummy matmuls on a zeroed scratch tile, spanning
            # the DMA-queue spin-up (~12us) at low energy ---
            scratch = c_pool.tile([128, 128], mybir.dt.bfloat16)
            nc.gpsimd.memset(scratch[:], 0.0)
            psw = psw_pool.tile([128, 64], mybir.dt.float32)
            for _ in range(N_WARM):
                nc.tensor.matmul(psw[:], scratch[:], scratch[:, 0:64],
                                 start=True, stop=True)

            # --- slice-0 packed weights + scales first on the sync queue ---
            qw16 = {}   # (ot, kb) -> int16 view [128, 1024]
            sc_t = {}   # (ot, kb) -> bf16 view [128, 512]

            def emit_wdma(ot, kb_order=range(KB), split_first=False):
                # one fused DMA per (ot, kb): packed words (cols 0-511) and
                # bf16 scale bits (cols 512-767) land together. split_first
                # lands the first kb's packed words ahead of its scales so
                # the first TENSOR_SCALAR can start sooner.
                for idx, kb in enumerate(kb_order):
                    q = qs_pool.tile([128, 768], mybir.dt.int32, tag="qs")
                    if split_first and idx == 0:
                        nc.sync.dma_start(q[:, 0:512], qws[ot, kb, :, 0:512])
                        nc.sync.dma_start(q[:, 512:768],
                                          qws[ot, kb, :, 512:768])
                    else:
                        nc.sync.dma_start(q[:], qws[ot, kb])
                    qw16[(ot, kb)] = q[:, 0:512].bitcast(mybir.dt.int16)
                    sc_t[(ot, kb)] = q[:, 512:768].bitcast(mybir.dt.bfloat16)

            # slice-0 startup consumes pairs q0.. in order; q0/q1 are kb0.
            emit_wdma(0, kb_order=(0, 1, 2, 3), split_first=True)
            # mean-term operands zero-padded to 128 K-rows: the rank-33 matmul
            # (32 group channels + the bias/ones row) then behaves like any
            # bf16 matmul (K-rows are free). DMA'd on the scalar engine's
            # queue so they land without queueing behind weights or x.
            xg_t = c_pool.tile([128, T], mybir.dt.bfloat16)
            s75_t = c_pool.tile([128, OS], mybir.dt.bfloat16)
            # zero the never-DMA'd upper rows on gpsimd (idle after its DMA
            # issues), emitted BEFORE the row-0..63 DMAs so the WAW ordering
            # doesn't gate the memsets on the transfers
            nc.gpsimd.memset(xg_t[64:128, :], 0.0)
            nc.gpsimd.memset(s75_t[64:128, :], 0.0)
            nc.scalar.dma_start(xg_t[0:64, :], xg)
            nc.scalar.dma_start(s75_t[0:64, :], s75)

            # --- o-sliced dequantization op streams (vector engine) ---
            # K-block kk = 8*kb + j; plane j lives in the int16 nibble-pair
            # extracted with shift s = 12 - 4*(j%4) at column parity
            # e = 1 (odd, j<4) or 0 (even, j>=4).
            wd = [[None] * NKB_BF for _ in range(OT)]
            w8 = [[None] * NPAIR for _ in range(OT)]

            DEFAULT_SCHED = tuple((kb, tuple(range(8))) for kb in range(KB))
            # slice 0: kb0 fp8 planes first (pairs q0,q1 feed the PE right
            # after warm-up), bf16 planes after; kb1..3 in (j, j+4) order so
            # each pair completes right after its nibble tile is extracted.
            SLICE0_SCHED = (
                (0, (4, 5, 6, 7, 0, 1, 2, 3)),
                (1, (0, 4, 1, 5, 2, 6, 3, 7)),
                (2, (0, 4, 1, 5, 2, 6, 3, 7)),
                (3, (0, 4, 1, 5, 2, 6, 3, 7)),
            )

            def slice_deq_ops(ot, sched=DEFAULT_SCHED):
                """Yield thunks emitting slice `ot`'s dequant (4xTS + 8 ops per kb)."""
                nibs_by_kb = {}

                def emit_ts(kb, nibs, shifts):
                    def go():
                        for s in shifts:
                            nib = nib_pool.tile([128, 1024], mybir.dt.int16,
                                                tag="nib", name=f"nib{s}")
                            nc.vector.tensor_scalar(
                                nib[:], qw16[(ot, kb)], s, 15,
                                op0=mybir.AluOpType.logical_shift_right,
                                op1=mybir.AluOpType.bitwise_and,
                            )
                            nibs[s] = nib
                    return go

                def emit_tt(j, kb, nibs):
                    def go():
                        s_ = 12 - 4 * (j % 4)
                        e = 1 if j < 4 else 0
                        pair = nibs[s_][:].rearrange("p (o e) -> p o e", e=2)
                        dest = _DEQ_DEST[(kb, j)]
                        if dest[0] == "bf":
                            w = w_pool.tile([128, 512], mybir.dt.bfloat16,
                                            tag="w")
                            nc.vector.tensor_tensor(
                                w[:], pair[:, :, e], sc_t[(ot, kb)],
                                op=mybir.AluOpType.mult)
                            wd[ot][dest[1]] = w
                        else:
                            _, q_, half = dest
                            if w8[ot][q_] is None:
                                w8[ot][q_] = w8_pool.tile(
                                    [128, 1024], mybir.dt.float8e4,
                                    tag="w8", name=f"w8_{ot}_{q_}")
                            # centered dequant: (nib - 7.5) * bf16(s*256)
                            nc.vector.scalar_tensor_tensor(
                                w8[ot][q_][:, ts(half, 512)],
                                pair[:, :, e], -CENTER, sc_t[(ot, kb)],
                                op0=mybir.AluOpType.add,
                                op1=mybir.AluOpType.mult)
                    return go

                for kb, js in sched:
                    if kb not in nibs_by_kb:
                        nibs_by_kb[kb] = {}
                        yield emit_ts(kb, nibs_by_kb[kb], (12, 8, 4, 0))
                    for j in js:
                        yield emit_tt(j, kb, nibs_by_kb[kb])

            def evict(ot, tt, ps):
                # bias already accumulated in PSUM via the mean-term matmul's
                # ones row; eviction is a pure scale on the idle scalar engine.
                # Output DMAs alternate queues so back-to-back evictions (and
                # the final pair at the kernel tail) don't serialize.
                ob = o_pool.tile([128, 512], mybir.dt.float32, tag="ob")
                nc.scalar.mul(ob[:], ps[:], 1.0 / WSCALE)
                nc.sync.dma_start(y[tt, :, ts(ot, 512)], ob[:])

            def corr_mm(ps, ot, tt, stop=False):
                nc.tensor.matmul(ps[:], xg_t[:, ts(tt, 128)],
                                 s75_t[:, ts(ot, 512)],
                                 start=False, stop=stop)

            def dr_mm1(ps, xt8_ap, ot, q_, start=False, stop=False):
                from concourse import mybir as mb
                lhs3 = xt8_ap[:, ts(q_, 256)].rearrange(
                    "p (two m) -> p two m", two=2)
                rhs3 = w8[ot][q_][:].rearrange(
                    "p (two o) -> p two o", two=2)
                nc.tensor.matmul(
                    ps[:], lhs3, rhs3, start=start, stop=stop,
                    perf_mode=mb.MatmulPerfMode.DoubleRow,
                )

            def dr_mms(ps, xt8_ap, ot, start=False):
                for q_ in range(NPAIR):
                    dr_mm1(ps, xt8_ap, ot, q_, start=(start and q_ == 0))

            # --- matmul: o-slice outer, t-tile inner ---
            # The first R0 t-tile groups of slice 0 run pair-major across R0
            # PSUM banks: each weight pair feeds R0 matmuls the moment the
            # dequant engines produce it, keeping the in-order PE busy during
            # slice-0 dequantization.
            xcs = [None] * R0
            xc8s = [None] * R0
            pss = []
            # gpsimd DMA issues FIRST in its program order (it also runs
            # dequant ops now); fp8 x first (DR phase starts the instant
            # warmup ends), then the small bf16 chunks.
            for g in range(R0):
                xc8 = xc8_pool.tile([128, NPAIR * 256], mybir.dt.float8e4,
                                    tag="xc8", name=f"xc8_{g}")
                nc.gpsimd.dma_start(xc8[:], xp8[g])
                xc8s[g] = xc8
            for g in range(R0):
                xc = xc_pool.tile([128, NKB_BF * 128], mybir.dt.bfloat16,
                                  tag="xc", name=f"xc{g}")
                nc.gpsimd.dma_start(xc[:], xp[g])
                xcs[g] = xc
            # slice-0 dequant emitted after the DMA issues above
            for op in slice_deq_ops(0, SLICE0_SCHED):
                op()
            for g in range(R0):
                pss.append(ps_pool.tile([128, 512], mybir.dt.float32,
                                        tag="ps", name=f"ps0_{g}"))
            # the mean-term operands land early on the scalar queue: the
            # rank-33 matmuls give the PE real work while the sync queue is
            # still delivering the first weight tiles
            for g in range(R0):
                nc.tensor.matmul(pss[g][:], xg_t[:, ts(g, 128)],
                                 s75_t[:, 0:512], start=True, stop=False)
            for q_ in range(NPAIR):
                for g in range(R0):
                    dr_mm1(pss[g], xc8s[g][:], 0, q_)
            for kk in range(NKB_BF):
                for g in range(R0):
                    nc.tensor.matmul(
                        pss[g][:], xcs[g][:, ts(kk, 128)], wd[0][kk][:],
                        start=False, stop=(kk == NKB_BF - 1),
                    )
            for g in range(R0):
                evict(0, g, pss[g])

            def bf_mms(ps, xt_bf, ot):
                # groups are started by the DR chain; never start here
                for kk in range(NKB_BF):
                    nc.tensor.matmul(
                        ps[:], xt_bf[:, ts(kk, 128)], wd[ot][kk][:],
                        start=False, stop=False,
                    )

            def load_x(tt, eng=None):
                # balance x traffic across both DMA queues: gpsimd saturates
                # during the slice-0 pairs loop if it carries every tile
                eng = eng or nc.gpsimd
                xt_bf = x_pool.tile([128, NKB_BF * 128], mybir.dt.bfloat16,
                                    tag="x", name=f"xt{tt}")
                eng.dma_start(xt_bf[:], xp[tt])
                xt8 = x8_pool.tile([128, NPAIR * 256], mybir.dt.float8e4,
                                   tag="x8t", name=f"xt8{tt}")
                nc.gpsimd.dma_start(xt8[:], xp8[tt])
                return xt_bf, xt8

            # --- slice 0, tiles R0..TT-1 in pairs (one DR bubble per pair),
            # interleaving slice 1+2 dequant between evictions ---
            emit_wdma(1)
            pending = list(slice_deq_ops(1)) + list(slice_deq_ops(2))
            n_single = (TT - R0) % 2
            n_pairs = (TT - R0) // 2
            per_pair = (len(pending) + n_pairs - 1) // n_pairs
            for pi in range(n_pairs):
                if pi == 3:
                    # slice-2 packed weights aren't consumed until the
                    # second half of the pairs loop; keep sync free early
                    emit_wdma(2)
                ta, tb = R0 + 2 * pi, R0 + 2 * pi + 1
                xa, xa8 = load_x(ta)
                xb, xb8 = load_x(tb, eng=nc.sync)
                psa = ps_pool.tile([128, 512], mybir.dt.float32, tag="ps",
                                   name=f"psa{pi}")
                psb = ps_pool.tile([128, 512], mybir.dt.float32, tag="ps",
                                   name=f"psb{pi}")
                # every PSUM group starts with its DR chain and every
                # normal-section entry follows a DR MM: all group starts and
                # mode transitions are the measured-free kind
                dr_mms(psa, xa8[:], 0, start=True)
                bf_mms(psa, xa, 0)
                corr_mm(psa, 0, ta, stop=True)
                dr_mms(psb, xb8[:], 0, start=True)
                bf_mms(psb, xb, 0)
                corr_mm(psb, 0, tb, stop=True)
                evict(0, ta, psa)
                evict(0, tb, psb)
                for _ in range(per_pair):
                    if pending:
                        pending.pop(0)()
            assert not pending
            for tt in range(TT - n_single, TT):
                # odd leftover slice-0 tile when (TT - R0) is odd
                xt_bf, xt8 = load_x(tt)
                pst = ps_pool.tile([128, 512], mybir.dt.float32, tag="ps",
                                   name=f"pst{tt}")
                dr_mms(pst, xt8[:], 0, start=True)
                bf_mms(pst, xt_bf, 0)
                corr_mm(pst, 0, tt, stop=True)
                evict(0, tt, pst)

            # --- slices 1+2 fused per t-tile: x loaded once, DR batched ---
            for tt in range(TT):
                xt_bf, xt8 = load_x(tt, eng=(nc.sync if tt % 2 else None))
                ps1 = ps_pool.tile([128, 512], mybir.dt.float32, tag="ps",
                                   name=f"ps1_{tt}")
                ps2 = ps_pool.tile([128, 512], mybir.dt.float32, tag="ps",
                                   name=f"ps2_{tt}")
                dr_mms(ps1, xt8[:], 1, start=True)
                bf_mms(ps1, xt_bf, 1)
                corr_mm(ps1, 1, tt, stop=True)
                dr_mms(ps2, xt8[:], 2, start=True)
                bf_mms(ps2, xt_bf, 2)
                corr_mm(ps2, 2, tt, stop=True)
                evict(1, tt, ps1)
                evict(2, tt, ps2)

    nc.compile()
    return nc


def _prep_inputs(x, qweight, scales, bias):
    bf16 = ml_dtypes.bfloat16
    fp8 = ml_dtypes.float8_e4m3
    # x -> K-permuted lhsT layout: XKK[tt, p, kk, m] = x[128*tt+m, i(kk, p)]
    # with i(kk=(kb,j), p) = 8*(128*kb + p) + j.
    xb = np.ascontiguousarray(x.T)                            # [I, T] f32
    xb = xb.reshape(KB, 128, 8, T).transpose(0, 2, 1, 3)      # [kb, j, p, t]
    xb = xb.reshape(NKB, 128, TT, 128).transpose(2, 1, 0, 3)  # [tt, p, kk, m]
    xkk = np.ascontiguousarray(xb)                            # f32
    # bf16 part: kk 0..NKB_BF-1
    xp = xkk[:, :, :NKB_BF, :].astype(bf16).reshape(TT, 128, NKB_BF * 128)
    # fp8 DoubleRow pairs
    x8l = [xkk[:, :, [a, b], :] for (a, b) in PAIRS]          # [tt,p,2,128] each
    x8 = np.stack(x8l, axis=2)                                # [tt,p,np,2,128]
    xp8 = np.ascontiguousarray(x8).astype(fp8).reshape(TT, 128, NPAIR * 256)

    # qweight -> [o-slice, kb, p, 512] per-core shards (packed words -- 4
    # bits/weight on the wire; nibble extraction on-device on the vector
    # engine where int16 shift/and runs at 2x rate)
    qwt = np.ascontiguousarray(qweight.T).reshape(KB, 128, O)

    # SC[kb, p, o] = bf16(scales[o, 8*kb + p//16] * WSCALE). The mean term
    # uses the SAME bf16 scale so the centering offset cancels exactly.
    st = np.ascontiguousarray(scales.T) * np.float32(WSCALE)  # [32, O]
    scp = np.repeat(st.reshape(KB, 8, O), 16, axis=1).astype(bf16)

    # --- rank-33 mean term (host side) ---
    # channels 0..31: exact x group sums over fp8 planes; channel 32: ones
    # row carrying bias*WSCALE. ch = 8*kb + G == scale group.
    xv = x.reshape(T, KB, 128, 8)                             # [t, kb, p, j]
    xg_host = np.zeros((64, T), dtype=bf16)                   # K-padded
    for kb in range(KB):
        js = [j for j in range(8) if _DEQ_DEST[(kb, j)][0] == "f8"]
        part = xv[:, kb, :, js].sum(axis=0)                   # [t, p]
        gsum = part.reshape(T, 8, 16).sum(axis=2)             # [t, G]
        xg_host[8 * kb:8 * kb + 8, :] = (
            np.ascontiguousarray(gsum.T).astype(bf16))
    xg_host[NCH, :] = bf16(1.0)
    # s75[ch, o] = 7.5 * bf16(scales * WSCALE); row NCH = bias * WSCALE
    scbf = np.ascontiguousarray(st.astype(bf16).astype(np.float32))
    s75_host = np.zeros((64, O), dtype=bf16)                  # K-padded
    s75_host[:NCH] = (np.float32(CENTER) * scbf).astype(bf16)
    s75_host[NCH] = (bias * np.float32(WSCALE)).astype(bf16)

    in_maps = []
    for c in range(NCORES):
        sl = slice(c * OS, (c + 1) * OS)
        qc = qwt[:, :, sl].reshape(KB, 128, OT, 512).transpose(2, 0, 1, 3)
        scc = scp[:, :, sl].reshape(KB, 128, OT, 512).transpose(2, 0, 1, 3)
        scc32 = np.ascontiguousarray(scc).view(np.uint16).reshape(
            OT, KB, 128, 512).view(np.uint32).view(np.int32)  # [.., 256]
        fused = np.concatenate([np.ascontiguousarray(qc), scc32], axis=3)
        in_maps.append({
            "xp": xp,
            "xp8": xp8,
            "qws": fused,
            "xg": xg_host,
            "s75": np.ascontiguousarray(s75_host[:, sl]),
        })
    return in_maps


def kernel(x, qweight, scales, bias):
    global _nc_cache, LAST_RESULTS
    from concourse.bass_utils import run_bass_kernel_spmd

    x = np.asarray(x, dtype=np.float32)
    qweight = np.asarray(qweight, dtype=np.int32)
    scales = np.asarray(scales, dtype=np.float32)
    bias = np.asarray(bias, dtype=np.float32)

    if _nc_cache is None:
        _nc_cache = _build_module()
    nc = _nc_cache

    in_maps = _prep_inputs(x, qweight, scales, bias)
    res = None
    for attempt in range(3):
        try:
            res = run_bass_kernel_spmd(nc, in_maps,
                                       core_ids=list(range(NCORES)))
            break
        except Exception:
            if attempt == 2:
                raise
    LAST_RESULTS = res
    return np.concatenate(
        [r["y"].reshape(T, OS) for r in res.results], axis=1)
